# revision 1
# baseline (speedup 1.0000x reference)
"""AdderNet layer (L1-distance "matmul" + bias scales + LayerNorm) on 8 TRN2 cores.

out[n, o] = LN(-sum_i |x[n,i]*bias_in[i] - w[i,o]| * bias_out[o])

Strategy (hardcoded for x:[2,2048,512], w:[512,512]):
  - Data-parallel over the 4096 flattened tokens: 512 tokens per core, weight
    replicated, no collectives. Host transposes each x-shard to [Cin, tok].
  - Per token: DVE/ScalarE produce D_c = |w_c - x_t| tiles ([128 cin, 512 out],
    bf16) for the 4 cin chunks; TensorE reduces over cin partitions with a
    ones-vector matmul, accumulating the 4 chunks into one PSUM row [1, 512].
  - PSUM rows are gathered 4-at-a-time into SBUF [128 tok, 512 out] tiles by
    DMA, then bias_out/LayerNorm run on the free (out) axis.
"""

import functools

import numpy as np
import ml_dtypes

N_CORES = 8
CIN = 512
COUT = 512
NTOK = 4096  # 2*2048 flattened tokens
TOK_PER_CORE = NTOK // N_CORES  # 512
NCHUNK = CIN // 128  # 4
NGROUP = TOK_PER_CORE // 128  # 4
EPS = 1e-5

# Which cin-chunk ScalarE produces (for 2 of 3 tokens; the rest go to DVE).
# Negative disables the ScalarE production path.
SCALAR_CHUNK = 3
# Tokens per PSUM tile (PSUM rows staged + scattered to SBUF per DMA).
TOK_PER_PSUM = 4


@functools.lru_cache(maxsize=1)
def _build_nc(debug_tokens=None, debug_skip=()):
    import concourse.bacc as bacc
    import concourse.mybir as mybir
    from concourse.tile import TileContext

    f32 = mybir.dt.float32
    bf16 = mybir.dt.bfloat16
    Alu = mybir.AluOpType
    Act = mybir.ActivationFunctionType

    nc = bacc.Bacc(
        "TRN2",
        debug=False,
        enable_asserts=False,
        target_bir_lowering=False,
        num_devices=N_CORES,
    )

    xT = nc.dram_tensor("xT", [CIN, TOK_PER_CORE], f32, kind="ExternalInput").ap()
    w = nc.dram_tensor("w", [CIN, COUT], bf16, kind="ExternalInput").ap()
    bias_in = nc.dram_tensor("bias_in", [CIN, 1], f32, kind="ExternalInput").ap()
    nbout_b = nc.dram_tensor("nbout_b", [128, COUT], f32, kind="ExternalInput").ap()
    gamma_b = nc.dram_tensor("gamma_b", [128, COUT], f32, kind="ExternalInput").ap()
    beta_b = nc.dram_tensor("beta_b", [128, COUT], f32, kind="ExternalInput").ap()
    y = nc.dram_tensor("y", [TOK_PER_CORE, COUT], f32, kind="ExternalOutput").ap()

    with TileContext(nc) as tc:
        with (
            tc.tile_pool(name="const", bufs=1) as cpool,
            tc.tile_pool(name="dtiles", bufs=10) as dpool,
            tc.tile_pool(name="psum", bufs=8, space="PSUM") as ppool,
            tc.tile_pool(name="sgrp", bufs=2) as spool,
            tc.tile_pool(name="ln", bufs=2) as lpool,
            tc.tile_pool(name="dram", bufs=2, space="DRAM") as drampool,
        ):
            # ---- constants / weights ----
            # |w - x| = 2*relu(w - x) - (w - x). The hot loop only computes
            # relu(w - x); the "- (w - x)" part telescopes into
            # colsum_w[o] - rowsum_x[t], both computed once below.
            twos = cpool.tile([128, 1], bf16, tag="twos")
            nc.vector.memset(twos, 2.0)
            ones_bf = cpool.tile([128, 1], bf16, tag="ones_bf")
            nc.vector.memset(ones_bf, 1.0)
            ones_f = cpool.tile([128, 1], f32, tag="ones_f")
            nc.vector.memset(ones_f, 1.0)
            ones_row = cpool.tile([1, 128], f32, tag="ones_row")
            nc.vector.memset(ones_row, 1.0)

            w_c = []
            for c in range(NCHUNK):
                wt = cpool.tile([128, COUT], bf16, tag=f"w{c}")
                nc.sync.dma_start(wt, w[c * 128 : (c + 1) * 128, :])
                w_c.append(wt)

            bin_c = []
            for c in range(NCHUNK):
                bt = cpool.tile([128, 1], f32, tag=f"bin{c}")
                nc.sync.dma_start(bt, bias_in[c * 128 : (c + 1) * 128, :])
                bin_c.append(bt)

            nb_t = cpool.tile([128, COUT], f32, tag="nb")
            nc.sync.dma_start(nb_t, nbout_b[:, :])
            ga_t = cpool.tile([128, COUT], f32, tag="ga")
            nc.sync.dma_start(ga_t, gamma_b[:, :])
            be_t = cpool.tile([128, COUT], f32, tag="be")
            nc.sync.dma_start(be_t, beta_b[:, :])

            # x^T chunks, scaled by bias_in (and negated copy for ScalarE bias)
            xs_c = []
            xneg_c = []
            for c in range(NCHUNK):
                xr = cpool.tile([128, TOK_PER_CORE], f32, tag=f"xr{c}")
                nc.sync.dma_start(xr, xT[c * 128 : (c + 1) * 128, :])
                xs = cpool.tile([128, TOK_PER_CORE], f32, tag=f"xs{c}")
                nc.vector.tensor_scalar(xs, xr, bin_c[c][:, 0:1], None, Alu.mult)
                xs_c.append(xs)
                if SCALAR_CHUNK >= 0:
                    xn = cpool.tile([128, TOK_PER_CORE], f32, tag=f"xn{c}")
                    nc.vector.tensor_scalar(xn, xs, -1.0, None, Alu.mult)
                    xneg_c.append(xn)
                else:
                    xneg_c.append(None)

            # ---- one-time: rowsum_x[t] and colsum_w[o] ----
            # rowsum_x: ones^T @ xs -> psum row [1, TOK], scatter to [128, NGROUP]
            ps_row = ppool.tile([1, TOK_PER_CORE], f32, tag="ps")
            for c in range(NCHUNK):
                nc.tensor.matmul(
                    ps_row[0:1, 0:TOK_PER_CORE], ones_f, xs_c[c],
                    start=(c == 0), stop=(c == NCHUNK - 1),
                )
            rowx_stage = cpool.tile([1, TOK_PER_CORE], f32, tag="rowx_stage")
            nc.scalar.copy(rowx_stage, ps_row[0:1, 0:TOK_PER_CORE])
            # Bounce through DRAM: SBUF-side free->partition rearranges are
            # not expressible (partition is physical); DRAM is linear.
            rowx_dram = drampool.tile([1, TOK_PER_CORE], f32, tag="rowx_dram")
            nc.sync.dma_start(rowx_dram, rowx_stage)
            rowx = cpool.tile([128, NGROUP], f32, tag="rowx")
            nc.sync.dma_start(
                rowx, rowx_dram.rearrange("p (g t) -> (p t) g", g=NGROUP)
            )

            # colsum_w: ones^T @ w -> [1, COUT], then broadcast to [128, COUT]
            ps_col = ppool.tile([1, COUT], f32, tag="ps")
            for c in range(NCHUNK):
                nc.tensor.matmul(
                    ps_col[0:1, 0:COUT], ones_bf, w_c[c],
                    start=(c == 0), stop=(c == NCHUNK - 1),
                )
            colw_row = cpool.tile([1, COUT], f32, tag="colw_row")
            nc.scalar.copy(colw_row, ps_col[0:1, 0:COUT])
            ps_bc = ppool.tile([128, COUT], f32, tag="ps")
            nc.tensor.matmul(ps_bc, ones_row, colw_row, start=True, stop=True)
            colw_b = cpool.tile([128, COUT], f32, tag="colw_b")
            nc.scalar.copy(colw_b, ps_bc)

            # ---- main loop ----
            for g in range(NGROUP):
                if debug_tokens is not None and g * 128 >= debug_tokens:
                    break
                s_g = spool.tile([128, COUT], f32, tag="sgrp")
                for t0 in range(0, 128, TOK_PER_PSUM):
                    if debug_tokens is not None and g * 128 + t0 >= debug_tokens:
                        continue
                    # TOK_PER_PSUM tokens run concurrently in distinct PE
                    # column groups: token tt's [1,512] reduction lands on
                    # PSUM partition 32*tt of one shared bank.
                    ps4 = ppool.tile([128, COUT], f32, tag="ps")
                    d = [[None] * NCHUNK for _ in range(TOK_PER_PSUM)]
                    for tt in range(TOK_PER_PSUM):
                        t = g * 128 + t0 + tt
                        for c in range(NCHUNK):
                            dt = dpool.tile([128, COUT], bf16, tag="d")
                            if SCALAR_CHUNK == c and (t % 3) != 0:
                                nc.scalar.activation(
                                    dt, w_c[c], Act.Relu,
                                    bias=xneg_c[c][:, t : t + 1], scale=1.0,
                                )
                            else:
                                nc.vector.tensor_scalar(
                                    dt, w_c[c], xs_c[c][:, t : t + 1], 0.0,
                                    Alu.subtract, Alu.max,
                                )
                            d[tt][c] = dt
                    # c-outer order: consecutive matmuls hit different column
                    # groups so their rhs streams overlap on the PE array.
                    for c in range(NCHUNK):
                        for tt in range(TOK_PER_PSUM):
                            nc.tensor.matmul(
                                ps4[32 * tt : 32 * tt + 1, :], twos, d[tt][c],
                                start=(c == 0), stop=(c == NCHUNK - 1),
                                tile_position=(0, 32 * tt),
                            )
                    # Strided ScalarE copy (partitions {0,32,64,96} stay in
                    # place), then a partition-scatter SBUF->SBUF DMA packs
                    # the 4 token rows into s_g.
                    # Full-tile copy: cycles scale with free-dim only, so
                    # copying all 128 partitions costs the same as 4; the
                    # DMA then gathers the 4 live rows (DMA may stride
                    # partitions, engines may not).
                    stg = dpool.tile([128, COUT], f32, tag="stg")
                    nc.scalar.copy(stg, ps4)
                    nc.sync.dma_start(
                        s_g[t0 : t0 + TOK_PER_PSUM, :], stg[0:128:32, :]
                    )

                if "ln" in debug_skip:
                    nc.sync.dma_start(y[g * 128 : (g + 1) * 128, :], s_g)
                    continue
                # ---- bias_out + LayerNorm over the free (out) axis ----
                # adder = -(s_g - colw + rowx); pre = adder*bias_out
                t0 = lpool.tile([128, COUT], f32, tag="t0")
                nc.vector.scalar_tensor_tensor(
                    t0, s_g, rowx[:, g : g + 1], colw_b, Alu.add, Alu.subtract
                )
                pre = lpool.tile([128, COUT], f32, tag="pre")
                nc.vector.tensor_tensor(pre, t0, nb_t, Alu.mult)
                if "ln1" in debug_skip:
                    nc.sync.dma_start(y[g * 128 : (g + 1) * 128, :], pre)
                    continue
                msum = lpool.tile([128, 1], f32, tag="msum")
                nc.vector.reduce_sum(msum, pre, axis=mybir.AxisListType.X)
                mean = lpool.tile([128, 1], f32, tag="mean")
                nc.vector.tensor_scalar(mean, msum, 1.0 / COUT, None, Alu.mult)
                cent = lpool.tile([128, COUT], f32, tag="cent")
                nc.vector.tensor_scalar(cent, pre, mean[:, 0:1], None, Alu.subtract)
                if "ln2" in debug_skip:
                    nc.sync.dma_start(y[g * 128 : (g + 1) * 128, :], cent)
                    continue
                sq = lpool.tile([128, COUT], f32, tag="sq")
                vsum = lpool.tile([128, 1], f32, tag="vsum")
                nc.scalar.activation(
                    sq, cent, Act.Square, accum_out=vsum
                )
                veps = lpool.tile([128, 1], f32, tag="veps")
                nc.vector.tensor_scalar(
                    veps, vsum, 1.0 / COUT, EPS, Alu.mult, Alu.add
                )
                if "ln3" in debug_skip:
                    nc.sync.dma_start(y[g * 128 : (g + 1) * 128, :], sq)
                    continue
                sstd = lpool.tile([128, 1], f32, tag="sstd")
                nc.scalar.sqrt(sstd, veps)
                rstd = lpool.tile([128, 1], f32, tag="rstd")
                nc.vector.reciprocal(rstd, sstd)
                t1 = lpool.tile([128, COUT], f32, tag="t1")
                nc.vector.tensor_scalar(t1, cent, rstd[:, 0:1], None, Alu.mult)
                if "ln4" in debug_skip:
                    nc.sync.dma_start(y[g * 128 : (g + 1) * 128, :], t1)
                    continue
                t2 = lpool.tile([128, COUT], f32, tag="t2")
                nc.vector.tensor_tensor(t2, t1, ga_t, Alu.mult)
                yt = lpool.tile([128, COUT], f32, tag="yt")
                nc.vector.tensor_tensor(yt, t2, be_t, Alu.add)
                nc.sync.dma_start(y[g * 128 : (g + 1) * 128, :], yt)

    nc.finalize()
    return nc


def _prep_inputs(x, weight, multi_bias_in, multi_bias_out, ln_gamma, ln_beta):
    x2 = np.asarray(x, np.float32).reshape(NTOK, CIN)
    w_bf = np.asarray(weight, np.float32).astype(ml_dtypes.bfloat16)
    bin_col = np.ascontiguousarray(
        np.asarray(multi_bias_in, np.float32).reshape(CIN, 1)
    )
    nbout = np.ascontiguousarray(
        np.broadcast_to(-np.asarray(multi_bias_out, np.float32).reshape(1, COUT), (128, COUT))
    )
    gab = np.ascontiguousarray(
        np.broadcast_to(np.asarray(ln_gamma, np.float32).reshape(1, COUT), (128, COUT))
    )
    beb = np.ascontiguousarray(
        np.broadcast_to(np.asarray(ln_beta, np.float32).reshape(1, COUT), (128, COUT))
    )
    in_maps = []
    for k in range(N_CORES):
        shard = x2[k * TOK_PER_CORE : (k + 1) * TOK_PER_CORE, :]  # [tok, cin]
        xTs = np.ascontiguousarray(shard.T)  # [cin, tok]
        in_maps.append(
            {
                "xT": xTs,
                "w": w_bf,
                "bias_in": bin_col,
                "nbout_b": nbout,
                "gamma_b": gab,
                "beta_b": beb,
            }
        )
    return in_maps


def _run(in_maps, trace=False, trace_cores=None):
    from concourse import bass_utils

    nc = _build_nc()
    return bass_utils.run_bass_kernel_spmd(
        nc,
        in_maps,
        core_ids=list(range(N_CORES)),
        trace=trace,
        trace_cores=trace_cores,
    )


def kernel(x, weight, multi_bias_in, multi_bias_out, ln_gamma, ln_beta):
    in_maps = _prep_inputs(x, weight, multi_bias_in, multi_bias_out, ln_gamma, ln_beta)
    res = _run(in_maps)
    out = np.concatenate([r["y"] for r in res.results], axis=0)
    return out.reshape(np.asarray(x).shape[:-1] + (COUT,)).astype(np.float32)

